# revision 18
# baseline (speedup 1.0000x reference)
"""Multi-head self-attention with positional bias, sharded over 8 NeuronCores.

Sharding: head-parallel. Core h computes head h for all batches; the full
output is the sum of the 8 per-core partials (row-parallel Wout), summed on
host in fp32.

v2 design (driven by the TimelineSim cost model, where a matmul costs
out_free_size * pe_cycle and engine element ops cost free_size * cycle_t):
  - everything bf16 on the wires (qT, bias, weights, pexp, oT, out); fp32
    only in PSUM accumulation and the exp input.
  - Wq/Wk merged into one [d, 128] projection matmul (halves proj MM count).
  - scores computed transposed ST[j, i] = k_j . q_i + bias[i, j]; the bias
    lands via EITHER an identity matmul on PE (start=True) OR a DVE
    scalar_tensor_tensor add staged through SBUF -- split by ALPHA to balance
    the PE and DVE engines.
  - exp on ACT (the hard floor: ~133us for 16.8M elements), 1024-wide ops.
  - softmax denominator: ones column 64 in v (costs nothing extra on PE);
    the oT evacuation keeps the den row in the same bf16 tile; a SBUF->SBUF
    transpose DMA turns den rows into per-token-tile columns for reciprocal.
  - loop order (ip, pair, lb) so only 2 oT accumulators are live -> PSUM fits
    st double-buffering (4) + ot (2) + out-proj po (2) = 8 banks.
  - out-projection + normalization + store pipelined per (ip, pair, lb).
"""

import numpy as np
import ml_dtypes
from contextlib import ExitStack

import concourse.bass as bass
import concourse.bacc as bacc
import concourse.mybir as mybir
import concourse.tile as tile
from concourse.bass_utils import run_bass_kernel_spmd
from concourse.masks import make_identity

HEADS = 8
DH = 64
B, N, D = 4, 2048, 512
SCALE = DH ** -0.5
N_CORES = 8

# fraction of (jt, lb) score tiles whose bias-add runs as a PE identity
# matmul; the rest run as DVE adds staged through SBUF.
ALPHA = 0.275
# fraction of output normalizations routed to ACT (rest DVE)
GAMMA = 0.5
# fraction of PSUM->SBUF evacuations (proj qk, oT) routed to ACT (rest DVE)
EVAC_ACT = 0.12

F32 = mybir.dt.float32
BF16 = mybir.dt.bfloat16
BF16NP = ml_dtypes.bfloat16


def build_nc(b=B, n=N, d=D, alpha=None, n_cores=1):
    """Build the per-core Bass program (SPMD; per-head data via inputs)."""
    if alpha is None:
        alpha = ALPHA
    assert b % 2 == 0 and n % 1024 == 0 and d % 128 == 0
    T = b * n
    CC = d // 128        # contraction chunks for projections
    NJ = n // 128        # key tiles (j)
    NIP = n // 1024      # i-windows of 1024
    NPAIR = b // 2
    VW = 65              # v block width (ones column at 64)

    nc = bacc.Bacc("TRN2", target_bir_lowering=False, debug=False,
                   num_devices=n_cores)
    qT = nc.declare_dram_parameter("qT", [d, T], BF16, isOutput=False)
    biasT = nc.declare_dram_parameter("biasT", [n, n], BF16, isOutput=False)
    wqk = nc.declare_dram_parameter("wqk", [d, 128], BF16, isOutput=False)
    wv = nc.declare_dram_parameter("wv", [d, DH], BF16, isOutput=False)
    wout = nc.declare_dram_parameter("wout", [DH, d], BF16, isOutput=False)
    out = nc.declare_dram_parameter("out", [T, d], BF16, isOutput=True)

    with ExitStack() as ctx:
        tc = ctx.enter_context(tile.TileContext(nc))

        const = ctx.enter_context(tc.tile_pool(name="const", bufs=1))
        qk_pool = ctx.enter_context(tc.tile_pool(name="qkT", bufs=1))
        v_pool = ctx.enter_context(tc.tile_pool(name="v", bufs=1))
        ot_sb_pool = ctx.enter_context(tc.tile_pool(name="ot_sb", bufs=1))
        bias_pool = ctx.enter_context(tc.tile_pool(name="bias", bufs=1))
        s_pool = ctx.enter_context(tc.tile_pool(name="s_sb", bufs=3))
        p_pool = ctx.enter_context(tc.tile_pool(name="pexp", bufs=4))
        out_pool = ctx.enter_context(tc.tile_pool(name="osb", bufs=6))

        ident_f32 = const.tile([128, 128], F32, tag="ident_f32")
        make_identity(nc, ident_f32)
        ident = const.tile([128, 128], BF16, tag="ident")
        nc.vector.tensor_copy(ident, ident_f32)
        zbias = const.tile([128, 1], F32, tag="zbias")
        nc.vector.memset(zbias, 0.0)

        wqk_sb = const.tile([128, CC, 128], BF16, tag="wqk")
        nc.sync.dma_start(out=wqk_sb, in_=wqk[:, :].rearrange("(c p) e -> p c e", p=128))
        wv_sb = const.tile([128, CC, DH], BF16, tag="wv")
        nc.sync.dma_start(out=wv_sb, in_=wv[:, :].rearrange("(c p) e -> p c e", p=128))
        wout_sb = const.tile([64, d], BF16, tag="wout")
        nc.sync.dma_start(out=wout_sb, in_=wout[:, :])

        qT_sb = [qk_pool.tile([128, n], BF16, tag=f"qT{p}", name=f"qT{p}") for p in range(NPAIR)]
        kT_sb = [qk_pool.tile([128, n], BF16, tag=f"kT{p}", name=f"kT{p}") for p in range(NPAIR)]
        v_sb = [v_pool.tile([128, NJ, VW], BF16, tag=f"v{bb}", name=f"v{bb}") for bb in range(b)]
        for bb in range(b):
            nc.vector.memset(v_sb[bb][:, :, DH:VW], 1.0)
        # oT + den row, per (pair, lb): rows 0..63 = oT (dh), row 64 = denom
        ot65 = [[ot_sb_pool.tile([VW, n], BF16, tag=f"ot{p}{l}", name=f"ot{p}{l}")
                 for l in range(2)] for p in range(NPAIR)]
        den_in = [const.tile([128, NJ], BF16, tag=f"den_in{bb}", name=f"di{bb}")
                  for bb in range(b)]
        den_dram = [nc.dram_tensor(f"den_dram{bb}", [n], BF16) for bb in range(b)]
        den_f32 = [const.tile([128, NJ], F32, tag=f"den_f32{bb}", name=f"df{bb}")
                   for bb in range(b)]
        recip_sb = [const.tile([128, NJ], F32, tag=f"recip{bb}", name=f"rc{bb}")
                    for bb in range(b)]

        # ---------------- qT tiles + projections (all batches) ----------------
        # DMA order: qt(bb0), qt(bb1), bias(ip0), qt(bb2), qt(bb3), bias(ip1)
        # so the first score block can start as soon as bb0/bb1 are projected.
        bias_t = {}

        def load_bias(ip, jqs=None):
            for jq in jqs if jqs is not None else range(NJ // 4):
                t = bias_pool.tile([128, 4, 1024], BF16, tag=f"bias{ip}{jq}",
                                   name=f"bias{ip}{jq}")
                nc.sync.dma_start(
                    out=t,
                    in_=biasT[jq * 512:(jq + 1) * 512,
                              ip * 1024:(ip + 1) * 1024].rearrange(
                                  "(c p) i -> p c i", p=128))
                bias_t[(ip, jq)] = t

        # preload the exp table set during the lead-in
        warm = const.tile([1, 1], BF16, tag="warm")
        nc.scalar.activation(warm, zbias[0:1, :], mybir.ActivationFunctionType.Exp,
                             bias=zbias[0:1, :])

        evac_quota = [0.0]

        def evac(dst, src):
            """PSUM->SBUF copy on DVE or ACT, balancing by EVAC_ACT."""
            evac_quota[0] += EVAC_ACT
            if evac_quota[0] >= 1.0:
                evac_quota[0] -= 1.0
                nc.scalar.copy(dst, src)
            else:
                nc.vector.tensor_copy(dst, src)

        # PSUM pools shared by every phase (8 banks exactly): st 2x[128,1024]
        # (4) + ot 2x[65,512] (2) + po 2x[128,512] (2). The projection phase
        # reuses the po ring so no pool-scope barrier separates phases.
        st_pool = ctx.enter_context(tc.tile_pool(name="st", bufs=2, space="PSUM"))
        ot_pool = ctx.enter_context(tc.tile_pool(name="ot", bufs=2, space="PSUM"))
        po_pool = ctx.enter_context(tc.tile_pool(name="po", bufs=2, space="PSUM"))
        qt_pool = ctx.enter_context(tc.tile_pool(name="qt", bufs=2 * CC))

        for bb in range(b):
            pair, lb = bb // 2, bb % 2
            rows = slice(64 * lb, 64 * lb + 64)
            qt_c = []
            for c in range(CC):
                t = qt_pool.tile([128, n], BF16, tag="qt", name="qtc")
                nc.sync.dma_start(out=t, in_=qT[c * 128:(c + 1) * 128,
                                                bb * n:(bb + 1) * n])
                qt_c.append(t)
            if bb == 0:
                load_bias(0, [0])
            elif bb == 1:
                load_bias(0, [1, 2, 3])
            for ic in range(n // 512):
                ps = po_pool.tile([128, 512], F32, tag="po")
                for c in range(CC):
                    nc.tensor.matmul(
                        ps, lhsT=wqk_sb[:, c, :],
                        rhs=qt_c[c][:, ic * 512:(ic + 1) * 512],
                        start=(c == 0), stop=(c == CC - 1))
                cols = slice(ic * 512, (ic + 1) * 512)
                evac(qT_sb[pair][rows, cols], ps[0:64, :])
                evac(kT_sb[pair][rows, cols], ps[64:128, :])
            for t8 in range(NJ // 8):
                psv = po_pool.tile([128, 512], F32, tag="po")
                psv8 = psv.rearrange("p (t e) -> p t e", e=DH)
                for k in range(8):
                    tt = t8 * 8 + k
                    for c in range(CC):
                        nc.tensor.matmul(
                            psv8[:, k, :],
                            lhsT=qt_c[c][:, tt * 128:(tt + 1) * 128],
                            rhs=wv_sb[:, c, :],
                            start=(c == 0), stop=(c == CC - 1))
                nc.vector.tensor_copy(
                    v_sb[bb][:, t8 * 8:(t8 + 1) * 8, 0:DH], psv8)
        load_bias(1)

        # ---------------- scores + softmax + P~^T V + out-proj ----------------
        pe_quota = [0.0]
        act_quota = [0.0]
        if True:

            def out_proj_tg(ip, pair, lb, tg):
                """One out-proj token tile: matmul + normalize + store."""
                bb = 2 * pair + lb
                tgg = ip * 8 + tg
                po = po_pool.tile([128, d], F32, tag="po")
                nc.tensor.matmul(
                    po, lhsT=ot65[pair][lb][0:64, tgg * 128:(tgg + 1) * 128],
                    rhs=wout_sb, start=True, stop=True)
                osb = out_pool.tile([128, d], BF16, tag="osb")
                act_quota[0] += GAMMA
                if act_quota[0] >= 1.0:
                    act_quota[0] -= 1.0
                    nc.scalar.mul(osb, po, recip_sb[bb][:, tgg:tgg + 1])
                else:
                    nc.vector.tensor_scalar_mul(
                        osb, po, recip_sb[bb][:, tgg:tgg + 1])
                nc.sync.dma_start(
                    out=out[bb * n + tgg * 128: bb * n + (tgg + 1) * 128, :],
                    in_=osb)

            pending = None
            for ip in range(NIP):
                for pair in range(NPAIR):
                    for lb in range(2):
                        bb = 2 * pair + lb
                        rows = slice(64 * lb, 64 * lb + 64)
                        ot_ps = [ot_pool.tile([VW, 512], F32, tag="ot", name="otp")
                                 for _ in range(2)]
                        for jt in range(NJ):
                            # spread the previous block's out-projection over
                            # this block's score iterations (one tile per jt)
                            if pending is not None and 4 <= jt < 12:
                                out_proj_tg(*pending, jt - 4)
                                if jt == 11:
                                    pending = None
                            bt = bias_t[(ip, jt // 4)][:, jt % 4, :]
                            st = st_pool.tile([128, 1024], F32, tag="st")
                            # bias path: PE identity-MM at block edges (so the
                            # st ring never waits on DVE backlog across block
                            # boundaries), else split by quota to balance
                            # PE vs DVE.
                            if jt in (0, 1, 14, 15):
                                use_pe = True
                            else:
                                pe_quota[0] += (16.0 * alpha - 4.0) / 12.0
                                use_pe = pe_quota[0] >= 1.0
                                if use_pe:
                                    pe_quota[0] -= 1.0
                            for il in range(2):
                                cols = slice(il * 512, (il + 1) * 512)
                                ic2 = ip * 2 + il
                                if use_pe:
                                    nc.tensor.matmul(
                                        st[:, cols], lhsT=ident, rhs=bt[:, cols],
                                        start=True, stop=False)
                                nc.tensor.matmul(
                                    st[:, cols],
                                    lhsT=kT_sb[pair][rows, jt * 128:(jt + 1) * 128],
                                    rhs=qT_sb[pair][rows, ic2 * 512:(ic2 + 1) * 512],
                                    start=not use_pe, stop=True)
                            if use_pe:
                                exp_in = st
                            else:
                                s_sb = s_pool.tile([128, 1024], F32, tag="s_sb")
                                nc.vector.scalar_tensor_tensor(
                                    s_sb, st, 0.0, bt,
                                    mybir.AluOpType.add, mybir.AluOpType.add)
                                exp_in = s_sb
                            pexp = p_pool.tile([128, 1024], BF16, tag="pexp")
                            nc.scalar.activation(
                                pexp, exp_in, mybir.ActivationFunctionType.Exp,
                                bias=zbias)
                            for il in range(2):
                                nc.tensor.matmul(
                                    ot_ps[il],
                                    lhsT=v_sb[bb][:, jt, :],
                                    rhs=pexp[:, il * 512:(il + 1) * 512],
                                    start=(jt == 0), stop=(jt == NJ - 1))
                        # evacuate oT (+den row) for this (ip, pair, lb)
                        for il in range(2):
                            ccols = slice((ip * 2 + il) * 512, (ip * 2 + il + 1) * 512)
                            evac(ot65[pair][lb][:, ccols], ot_ps[il])
                        # den row -> per-token-tile columns (via DRAM bounce);
                        # start the round-trip now, emit the out-proj later so
                        # its latency hides under the next block's score work.
                        nc.sync.dma_start(
                            out=den_dram[bb][ip * 1024:(ip + 1) * 1024],
                            in_=ot65[pair][lb][64:65, ip * 1024:(ip + 1) * 1024])
                        nc.sync.dma_start(
                            out=den_in[bb][:, ip * 8:(ip + 1) * 8],
                            in_=den_dram[bb][ip * 1024:(ip + 1) * 1024]
                            .rearrange("(t p) -> p t", p=128))
                        nc.gpsimd.tensor_copy(
                            den_f32[bb][:, ip * 8:(ip + 1) * 8],
                            den_in[bb][:, ip * 8:(ip + 1) * 8])
                        nc.vector.reciprocal(
                            recip_sb[bb][:, ip * 8:(ip + 1) * 8],
                            den_f32[bb][:, ip * 8:(ip + 1) * 8])
                        pending = (ip, pair, lb)
            act_quota[0] = 8.0  # tail: ACT is idle, put all norms there
            for tg in range(8):
                out_proj_tg(*pending, tg)
    nc.compile()
    return nc


def make_in_maps(query, pos_bias, Wq, Wk, Wv, Wout, n_cores=N_CORES):
    """Host-side sharding/layout prep. Head h -> core h."""
    query = np.asarray(query, dtype=np.float32)
    pos_bias = np.asarray(pos_bias, dtype=np.float32)
    Wq = np.asarray(Wq, dtype=np.float32)
    Wk = np.asarray(Wk, dtype=np.float32)
    Wv = np.asarray(Wv, dtype=np.float32)
    Wout = np.asarray(Wout, dtype=np.float32)

    b, n, d = query.shape
    qT = np.ascontiguousarray(query.reshape(b * n, d).T).astype(BF16NP)
    wq_s = Wq * np.float32(SCALE)
    in_maps = []
    for h in range(n_cores):
        sl = slice(h * DH, (h + 1) * DH)
        wqk = np.concatenate([wq_s[:, sl], Wk[:, sl]], axis=1)
        in_maps.append({
            "qT": qT,
            "biasT": np.ascontiguousarray(pos_bias[h].T).astype(BF16NP),
            "wqk": np.ascontiguousarray(wqk).astype(BF16NP),
            "wv": np.ascontiguousarray(Wv[:, sl]).astype(BF16NP),
            "wout": np.ascontiguousarray(Wout[sl, :]).astype(BF16NP),
        })
    return in_maps


def run_device(in_maps, b=B, n=N, d=D, trace=False, **kw):
    nc = build_nc(b, n, d, n_cores=len(in_maps))
    return run_bass_kernel_spmd(nc, in_maps, list(range(len(in_maps))), trace=trace, **kw)


def assemble(results, b=B, n=N, d=D):
    acc = np.zeros((b * n, d), dtype=np.float32)
    for r in results:
        acc += np.asarray(r["out"], dtype=np.float32)
    return acc.reshape(b, n, d)


def kernel(query, pos_bias, Wq, Wk, Wv, Wout):
    in_maps = make_in_maps(query, pos_bias, Wq, Wk, Wv, Wout)
    res = run_device(in_maps)
    return assemble(res.results)
